# revision 1
# baseline (speedup 1.0000x reference)
"""CenterLoss Trainium2 kernel (data-parallel over 8 NeuronCores).

loss = sum(clip(distmat * onehot(labels), 1e-12, 1e12)) / B,
distmat[i,c] = ||x_i - centers_c||^2. Only the (i, labels_i) entries survive
the mask; the B*(C-1) masked entries contribute exactly 1e-12 each (added
analytically on host). For this distribution d_i ~ 4096, so the clip never
binds and the sum decomposes exactly:

  sum_i d_i = sum_i ||x_i||^2 + sum_c n_c ||c_c||^2 - 2 sum_c <s_c, c_c>

with s = onehot(labels)^T @ x. The device computes s on the PE with fp8e4
DoubleRow matmuls (one-hot is exact 0/1 in fp8; x's fp8 rounding only
touches the small cross term — measured 9.4e-6 relative on HW). ||x||^2 and
||c||^2 stay exact f32 on the scalar engine. No center-row gather: HBM
traffic is 14.6 MB/core (x 8.4 + centers 6.2) instead of 16.8 MB, and the
-2<s,c> contraction is fused into the PSUM drain via scalar_tensor_tensor
(NOT tensor_tensor_reduce, which this runtime rejects, as does the N=1
DoubleRow count matmul — counts come from a host label histogram instead).

Sharding: x/labels split along batch (1024 samples/core), centers
replicated. Per-core output is a [128, 40] block of raw partial columns:
  cols 0..7   sum ||x_i||^2 per sample-tile (f32-exact)
  cols 8..31  -2*<s_mn, centers_mn> per (class-tile m, 512-col chunk n)
  cols 32..37 cn2: ||c_c||^2 for class c = m*128 + partition (f32-exact)
Host combine (f64): sum cols 0..31 over cores + <histogram(labels), cn2>
+ B*(C-1)*1e-12, divided by B.
"""

from contextlib import ExitStack

import numpy as np

import concourse.bacc as bacc
import concourse.tile as tile
from concourse import mybir
from concourse.bass_utils import run_bass_kernel_spmd

N_CORES = 8
B = 8192
D = 2048
C = 751
BS = B // N_CORES  # samples per core
P = 128
NT = BS // P       # sample tiles per core
M = 768            # classes padded to a multiple of 128
MT = M // P        # class tiles
KDR = NT // 2      # fp8 DoubleRow k-tiles (256 samples each)
NCH = D // 512     # feature chunks (one PSUM bank each)
OUTW = 40
FP8 = mybir.dt.float8e4
CLIP_LO = 1e-12

_NC = None


def build_nc():
    nc = bacc.Bacc("TRN2", target_bir_lowering=False)
    x = nc.dram_tensor("x", [BS, D], mybir.dt.float32, kind="ExternalInput")
    labels = nc.dram_tensor("labels", [P, NT], mybir.dt.int32, kind="ExternalInput")
    centers = nc.dram_tensor("centers", [C, D], mybir.dt.float32, kind="ExternalInput")
    out = nc.dram_tensor("partial", [P, OUTW], mybir.dt.float32, kind="ExternalOutput")

    # x_r[p, t, :] = x[t*128 + p, :]
    x_r = x[:].rearrange("(t p) d -> p t d", p=P)

    with tile.TileContext(nc) as tc, ExitStack() as ctx:
        xp = ctx.enter_context(tc.tile_pool(name="xp", bufs=3))
        sqp = ctx.enter_context(tc.tile_pool(name="sqp", bufs=2))
        cperm = ctx.enter_context(tc.tile_pool(name="cperm", bufs=1))
        perm = ctx.enter_context(tc.tile_pool(name="perm", bufs=1))
        psp = ctx.enter_context(tc.tile_pool(name="psp", bufs=8, space="PSUM"))

        # labels ride the ACT HWDGE ring so the x loads' SP ring is unblocked
        lab = perm.tile([P, NT], mybir.dt.int32)
        nc.scalar.dma_start(out=lab[:], in_=labels[:])
        lab_f = perm.tile([P, NT], mybir.dt.float32)
        nc.vector.tensor_copy(out=lab_f[:], in_=lab[:])

        iota_i = perm.tile([P, M], mybir.dt.int32)
        nc.gpsimd.iota(iota_i[:], pattern=[[1, M]], base=0, channel_multiplier=0)
        iota_f = perm.tile([P, M], mybir.dt.float32)
        nc.vector.tensor_copy(out=iota_f[:], in_=iota_i[:])

        out_sb = perm.tile([P, OUTW], mybir.dt.float32)
        nc.vector.memset(out_sb[:], 0.0)

        # fp8 DoubleRow-packed x and one-hot: [128, 2, *], j = sample tile 2k+j
        x8, oh8 = [], []
        for k in range(KDR):
            x8_k = perm.tile([P, 2, D], FP8, tag=f"x8_{k}")
            oh8_k = perm.tile([P, 2, M], FP8, tag=f"oh8_{k}")
            x8.append(x8_k)
            oh8.append(oh8_k)

        for t in range(NT):
            k, j = divmod(t, 2)
            xt = xp.tile([P, D], mybir.dt.float32, tag="xt")
            nc.sync.dma_start(out=xt[:], in_=x_r[:, t, :])
            sq = sqp.tile([P, D], mybir.dt.float32, tag="sq")
            nc.scalar.activation(
                out=sq[:], in_=xt[:], func=mybir.ActivationFunctionType.Square,
                accum_out=out_sb[:, t : t + 1],
            )
            nc.vector.tensor_copy(out=x8[k][:, j, :], in_=xt[:])
            nc.vector.tensor_scalar(
                out=oh8[k][:, j, :], in0=iota_f[:], scalar1=lab_f[:, t : t + 1],
                scalar2=None, op0=mybir.AluOpType.is_equal,
            )

        cts = []
        for m in range(MT):
            r0 = m * P
            rows = min(C - r0, P)
            ct = cperm.tile([P, D], mybir.dt.float32, tag=f"ct{m}")
            if rows < P:
                nc.vector.memset(ct[:], 0.0)  # pad rows must be finite zeros
            nc.sync.dma_start(out=ct[:rows, :], in_=centers[r0 : r0 + rows, :])
            sqc = sqp.tile([P, D], mybir.dt.float32, tag="sq")
            nc.scalar.activation(
                out=sqc[:rows, :], in_=ct[:rows, :],
                func=mybir.ActivationFunctionType.Square,
                accum_out=out_sb[:rows, 32 + m : 33 + m],
            )
            cts.append(ct)

        for m in range(MT):
            ps_row = []
            for _n in range(NCH):
                ps_n = psp.tile([P, 512], mybir.dt.float32, tag="ps")
                ps_row.append(ps_n)
            for k in range(KDR):
                lhs = oh8[k][:, :, m * P : (m + 1) * P]
                for n in range(NCH):
                    nc.tensor.matmul(
                        out=ps_row[n][:], lhsT=lhs,
                        rhs=x8[k][:, :, n * 512 : (n + 1) * 512],
                        start=(k == 0), stop=(k == KDR - 1),
                        perf_mode=mybir.MatmulPerfMode.DoubleRow,
                    )
            for n in range(NCH):
                stt_o = sqp.tile([P, 512], mybir.dt.float32, tag="stt_o")
                nc.vector.scalar_tensor_tensor(
                    out=stt_o[:], in0=ps_row[n][:], scalar=-2.0,
                    in1=cts[m][:, n * 512 : (n + 1) * 512],
                    op0=mybir.AluOpType.mult, op1=mybir.AluOpType.mult,
                    accum_out=out_sb[:, 8 + m * NCH + n : 9 + m * NCH + n],
                )

        nc.sync.dma_start(out=out[:], in_=out_sb[:])
    nc.compile()
    return nc


def make_in_maps(x, labels, centers):
    in_maps = []
    for k in range(N_CORES):
        xs = np.ascontiguousarray(x[k * BS : (k + 1) * BS])
        # lab[p, t] = labels_shard[t*P + p], matching the x tile layout
        ls = np.ascontiguousarray(labels[k * BS : (k + 1) * BS].reshape(NT, P).T)
        in_maps.append({"x": xs, "labels": ls, "centers": centers})
    return in_maps


def combine_partials(partials, labels):
    total = 0.0
    for p in partials:
        total += float(np.sum(p[:, :32].astype(np.float64)))
    # n_c * ||c_c||^2: label histogram (host index count) x device-computed cn2
    cn2 = partials[0][:, 32 : 32 + MT].astype(np.float64)  # class c = m*128+p
    hist = np.bincount(np.asarray(labels).astype(np.int64), minlength=M)
    total += float(np.sum(hist.reshape(MT, P).T * cn2))
    total += float(B) * float(C - 1) * CLIP_LO
    return np.array(total / B, dtype=np.float32)


def kernel(**inputs) -> np.ndarray:
    global _NC
    x = np.ascontiguousarray(np.asarray(inputs["x"], dtype=np.float32))
    labels = np.asarray(inputs["labels"]).astype(np.int32)
    centers = np.ascontiguousarray(np.asarray(inputs["centers"], dtype=np.float32))
    assert x.shape == (B, D) and labels.shape == (B,) and centers.shape == (C, D)

    if _NC is None:
        _NC = build_nc()
    res = run_bass_kernel_spmd(
        _NC, make_in_maps(x, labels, centers), core_ids=list(range(N_CORES))
    )
    return combine_partials([r["partial"] for r in res.results], labels)



# revision 6
# speedup vs baseline: 1.4569x; 1.4569x over previous
"""CenterLoss Trainium2 kernel (label-bucketed data-parallel over 8 cores).

loss = sum(clip(distmat * onehot(labels), 1e-12, 1e12)) / B,
distmat[i,c] = ||x_i - centers_c||^2. Only the (i, labels_i) entries survive
the mask; the B*(C-1) masked entries contribute exactly 1e-12 each (added
analytically on host). For this distribution d_i ~ 4096, so the clip never
binds and the sum decomposes exactly:

  sum_i d_i = sum_i ||x_i||^2 + sum_c n_c ||c_c||^2 - 2 sum_c <s_c, c_c>

with s = onehot(labels)^T @ x (computed per core over its local samples).

Sharding is the key bandwidth optimization: samples are SORTED BY LABEL on
host and split into 8 equal chunks of 1024. Each chunk spans a contiguous
label range (<= 128 classes for uniform labels), so each core only loads the
center rows its samples reference: ~0.8 MB instead of the full 6.15 MB
replica. Per-core HBM traffic drops from 14.6 MB to ~9.2 MB, which is the
roofline term (x itself is 8.4 MB/core and must be read once). Labels are
relabeled to the local window and the per-class count vector needed for the
n_c ||c_c||^2 term is a host-side histogram (as in the torch reference's
bincount), combined with the device-computed ||c_c||^2 column.

Device pipeline per core (M = 128*mt local classes, mt=1 for uniform data):
  - x arrives as 6 full [128, 2048] tiles + 2 tiles split into 4 512-col
    chunks (tiles 6, 7) so the end-of-stream compute tail is ~1.5 us, not 5.
  - ACT: per-tile Square with accum_out -> per-sample ||x_i||^2 columns;
    also ||c_c||^2 for the local window (early, while DMA streams).
  - DVE: fp32 -> fp8e4 copies of x tiles (exact one-hot is fp8), one-hot
    build, and the PSUM drain scalar_tensor_tensor (-2 * S) . C with
    accum_out per 512-col chunk.
  - PE: fp8 DoubleRow matmuls accumulate S = onehot^T x in 4 PSUM banks.

Per-core output is a [128, OUTW] block of raw partial columns; host combine
(f64) sums them, adds <histogram, cn2> and B*(C-1)*1e-12, divides by B.
"""

from contextlib import ExitStack

import numpy as np

import concourse.bacc as bacc
import concourse.tile as tile
from concourse import mybir
from concourse.bass_utils import run_bass_kernel_spmd

N_CORES = 8
B = 8192
D = 2048
C = 751
BS = B // N_CORES  # samples per core
P = 128
NT = BS // P       # sample tiles per core (8)
KDR = NT // 2      # fp8 DoubleRow k-tiles (256 samples each)
NCH = 4            # 512-col feature chunks (one PSUM bank each)
CH = D // NCH      # 512
FULL_T = NT - 2    # tiles DMA'd/squared whole; last 2 are column-chunked
FP8 = mybir.dt.float8e4
CLIP_LO = 1e-12

# Default geometry matches the reference's seed-0 data (max label span 99).
DEF_CR = 99
DEF_MT = 1

_NC_CACHE = {}


def _cols(mt):
    """Output column layout for a given class-tile count."""
    xsq = FULL_T + 2 * NCH           # 6 full-tile cols + 8 chunk cols = 14
    stt0 = xsq                       # mt*NCH cross-term cols
    cn0 = stt0 + mt * NCH            # mt ||c||^2 cols
    outw = -(-(cn0 + mt) // 4) * 4
    return stt0, cn0, outw


def build_nc(cr=DEF_CR, mt=DEF_MT):
    M = P * mt
    STT0, CN0, OUTW = _cols(mt)
    nc = bacc.Bacc("TRN2", target_bir_lowering=False)
    x = nc.dram_tensor("x", [BS, D], mybir.dt.float32, kind="ExternalInput")
    labels = nc.dram_tensor("labels", [P, NT], mybir.dt.int32, kind="ExternalInput")
    centers = nc.dram_tensor("centers", [cr, D], mybir.dt.float32, kind="ExternalInput")
    out = nc.dram_tensor("partial", [P, OUTW], mybir.dt.float32, kind="ExternalOutput")

    # x_r[p, t, :] = x[t*128 + p, :]
    x_r = x[:].rearrange("(t p) d -> p t d", p=P)

    with tile.TileContext(nc) as tc, ExitStack() as ctx:
        xp = ctx.enter_context(tc.tile_pool(name="xp", bufs=3))
        sqp = ctx.enter_context(tc.tile_pool(name="sqp", bufs=2))
        csq = ctx.enter_context(tc.tile_pool(name="csq", bufs=2))
        stp = ctx.enter_context(tc.tile_pool(name="stp", bufs=2))
        perm = ctx.enter_context(tc.tile_pool(name="perm", bufs=1))
        psp = ctx.enter_context(tc.tile_pool(name="psp", bufs=1, space="PSUM"))

        # labels ride the ACT HWDGE ring so the x loads' SP ring is unblocked
        lab = perm.tile([P, NT], mybir.dt.int32)
        nc.scalar.dma_start(out=lab[:], in_=labels[:])

        # centers window, one [<=128, D] tile per class tile, also ACT ring
        cts = []
        for m in range(mt):
            rows = min(cr - m * P, P)
            ct = perm.tile([P, D], mybir.dt.float32, tag=f"ct{m}")
            if rows < P:
                # pad rows must be finite zeros; partition start must be
                # 32-aligned, so memset a superset first, then DMA over it
                nc.gpsimd.memset(ct[(rows // 32) * 32 :, :], 0.0)
            nc.scalar.dma_start(out=ct[:rows, :], in_=centers[m * P : m * P + rows, :])
            cts.append(ct)

        iota_i = perm.tile([P, M], mybir.dt.int32)
        nc.gpsimd.iota(iota_i[:], pattern=[[1, M]], base=0, channel_multiplier=0)
        iota_f = perm.tile([P, M], mybir.dt.float32)
        nc.vector.tensor_copy(out=iota_f[:], in_=iota_i[:])
        lab_f = perm.tile([P, NT], mybir.dt.float32)
        nc.vector.tensor_copy(out=lab_f[:], in_=lab[:])

        out_sb = perm.tile([P, OUTW], mybir.dt.float32)
        nc.vector.memset(out_sb[:], 0.0)

        # fp8 DoubleRow-packed x and one-hot: [128, 2, *], j = sample tile 2k+j
        x8 = [perm.tile([P, 2, D], FP8, tag=f"x8_{k}", name=f"x8_{k}")
              for k in range(KDR)]
        oh8 = [perm.tile([P, 2, M], FP8, tag=f"oh8_{k}", name=f"oh8_{k}")
               for k in range(KDR)]
        for t in range(NT):
            k, j = divmod(t, 2)
            nc.vector.tensor_scalar(
                out=oh8[k][:, j, :], in0=iota_f[:], scalar1=lab_f[:, t : t + 1],
                scalar2=None, op0=mybir.AluOpType.is_equal,
            )

        # Full x tiles 0..FULL_T-1: DMA (SP ring) -> ACT square -> DVE fp8 copy
        for t in range(FULL_T):
            k, j = divmod(t, 2)
            xt = xp.tile([P, D], mybir.dt.float32, tag="xt")
            nc.sync.dma_start(out=xt[:], in_=x_r[:, t, :])
            sq = sqp.tile([P, D], mybir.dt.float32, tag="sq")
            nc.scalar.activation(
                out=sq[:], in_=xt[:], func=mybir.ActivationFunctionType.Square,
                accum_out=out_sb[:, t : t + 1],
            )
            if t == 0:
                # ||c||^2 early, in ACT's idle window right after ct lands
                for m in range(mt):
                    rows = min(cr - m * P, P)
                    sqc = sqp.tile([P, D], mybir.dt.float32, tag="sq")
                    nc.scalar.activation(
                        out=sqc[:rows, :], in_=cts[m][:rows, :],
                        func=mybir.ActivationFunctionType.Square,
                        accum_out=out_sb[:rows, CN0 + m : CN0 + m + 1],
                    )
            nc.vector.tensor_copy(out=x8[k][:, j, :], in_=xt[:])

        # Tiles 6 and 7 stream in 512-col chunks to shrink the compute tail.
        xtail = [perm.tile([P, D], mybir.dt.float32, tag=f"xt{t}", name=f"xtail{t}")
                 for t in (0, 1)]
        for i, t in enumerate((FULL_T, FULL_T + 1)):
            for n in range(NCH):
                sl = slice(n * CH, (n + 1) * CH)
                nc.sync.dma_start(out=xtail[i][:, sl], in_=x_r[:, t, sl])

        # ACT: chunk squares, one accum col each (issued after sq0..sq5)
        for i in range(2):
            for n in range(NCH):
                sl = slice(n * CH, (n + 1) * CH)
                cq = csq.tile([P, CH], mybir.dt.float32, tag="cq")
                nc.scalar.activation(
                    out=cq[:], in_=xtail[i][:, sl],
                    func=mybir.ActivationFunctionType.Square,
                    accum_out=out_sb[:, FULL_T + i * NCH + n : FULL_T + i * NCH + n + 1],
                )

        if mt <= 2:
            # Pipelined: all PSUM banks live, k=3 + drain per chunk as the
            # last tile's chunks arrive.
            ps = [[psp.tile([P, CH], mybir.dt.float32, tag=f"ps{m}_{n}",
                            name=f"ps{m}_{n}")
                   for n in range(NCH)] for m in range(mt)]
            for k in range(KDR - 1):
                for m in range(mt):
                    lhs = oh8[k][:, :, m * P : (m + 1) * P]
                    for n in range(NCH):
                        nc.tensor.matmul(
                            out=ps[m][n][:], lhsT=lhs,
                            rhs=x8[k][:, :, n * CH : (n + 1) * CH],
                            start=(k == 0), stop=False,
                            perf_mode=mybir.MatmulPerfMode.DoubleRow,
                        )
            # tile 6 chunks -> x8[3] row 0 (copies on DVE, in arrival order)
            for n in range(NCH):
                sl = slice(n * CH, (n + 1) * CH)
                nc.vector.tensor_copy(out=x8[KDR - 1][:, 0, sl], in_=xtail[0][:, sl])

            def drain(n):
                for m in range(mt):
                    so = stp.tile([P, CH], mybir.dt.float32, tag="so")
                    nc.vector.scalar_tensor_tensor(
                        out=so[:], in0=ps[m][n][:], scalar=-2.0,
                        in1=cts[m][:, n * CH : (n + 1) * CH],
                        op0=mybir.AluOpType.mult, op1=mybir.AluOpType.mult,
                        accum_out=out_sb[:, STT0 + m * NCH + n : STT0 + m * NCH + n + 1],
                    )

            # tile 7 chunk copies interleaved with final matmuls + drains
            for n in range(NCH):
                sl = slice(n * CH, (n + 1) * CH)
                nc.vector.tensor_copy(out=x8[KDR - 1][:, 1, sl], in_=xtail[1][:, sl])
                for m in range(mt):
                    nc.tensor.matmul(
                        out=ps[m][n][:],
                        lhsT=oh8[KDR - 1][:, :, m * P : (m + 1) * P],
                        rhs=x8[KDR - 1][:, :, sl],
                        start=False, stop=True,
                        perf_mode=mybir.MatmulPerfMode.DoubleRow,
                    )
                if n >= 1:
                    drain(n - 1)
            drain(NCH - 1)
        else:
            # Degenerate label distributions (> 256-class span): correctness-
            # first sequential per-class-tile accumulation, reusing banks.
            for n in range(NCH):
                sl = slice(n * CH, (n + 1) * CH)
                nc.vector.tensor_copy(out=x8[KDR - 1][:, 0, sl], in_=xtail[0][:, sl])
                nc.vector.tensor_copy(out=x8[KDR - 1][:, 1, sl], in_=xtail[1][:, sl])
            for m in range(mt):
                lhs_m = [oh8[k][:, :, m * P : (m + 1) * P] for k in range(KDR)]
                pr = [psp.tile([P, CH], mybir.dt.float32, tag=f"ps{n}", name="pr")
                      for n in range(NCH)]
                for k in range(KDR):
                    for n in range(NCH):
                        nc.tensor.matmul(
                            out=pr[n][:], lhsT=lhs_m[k],
                            rhs=x8[k][:, :, n * CH : (n + 1) * CH],
                            start=(k == 0), stop=(k == KDR - 1),
                            perf_mode=mybir.MatmulPerfMode.DoubleRow,
                        )
                for n in range(NCH):
                    so = stp.tile([P, CH], mybir.dt.float32, tag="so")
                    nc.vector.scalar_tensor_tensor(
                        out=so[:], in0=pr[n][:], scalar=-2.0,
                        in1=cts[m][:, n * CH : (n + 1) * CH],
                        op0=mybir.AluOpType.mult, op1=mybir.AluOpType.mult,
                        accum_out=out_sb[:, STT0 + m * NCH + n : STT0 + m * NCH + n + 1],
                    )

        nc.sync.dma_start(out=out[:], in_=out_sb[:])
    nc.compile()
    return nc


def _get_nc(cr, mt):
    key = (cr, mt)
    if key not in _NC_CACHE:
        _NC_CACHE[key] = build_nc(cr, mt)
    return _NC_CACHE[key]


def _shard(x, labels, centers):
    """Sort samples by label, split into 8 equal chunks, build per-core
    inputs with a local (relabeled) class window and its center rows."""
    order = np.argsort(labels, kind="stable")
    ls = labels[order]
    bases, spans = [], []
    for k in range(N_CORES):
        lo, hi = ls[k * BS], ls[(k + 1) * BS - 1]
        bases.append(int(lo))
        spans.append(int(hi - lo + 1))
    cr = max(spans)
    mt = -(-cr // P)
    in_maps, counts = [], []
    for k in range(N_CORES):
        idx = order[k * BS : (k + 1) * BS]
        xs = np.ascontiguousarray(x[idx])
        lloc = (ls[k * BS : (k + 1) * BS] - bases[k]).astype(np.int32)
        # lab[p, t] = lloc[t*P + p], matching the x tile layout
        lab = np.ascontiguousarray(lloc.reshape(NT, P).T)
        cw = np.zeros((cr, D), dtype=np.float32)
        cw[: spans[k]] = centers[bases[k] : bases[k] + spans[k]]
        in_maps.append({"x": xs, "labels": lab, "centers": cw})
        counts.append(np.bincount(lloc, minlength=mt * P).astype(np.float64))
    return in_maps, counts, cr, mt


def make_in_maps(x, labels, centers):
    return _shard(x, labels, centers)[0]


def _combine(partials, counts, mt):
    STT0, CN0, _ = _cols(mt)
    total = 0.0
    for k, p in enumerate(partials):
        p64 = p.astype(np.float64)
        total += float(np.sum(p64[:, :CN0]))
        # n_c * ||c_c||^2: host label histogram x device-computed ||c||^2
        cn2 = p64[:, CN0 : CN0 + mt]  # class m*128 + partition
        total += float(np.sum(counts[k].reshape(mt, P).T * cn2))
    total += float(B) * float(C - 1) * CLIP_LO
    return np.array(total / B, dtype=np.float32)


def kernel(**inputs) -> np.ndarray:
    x = np.ascontiguousarray(np.asarray(inputs["x"], dtype=np.float32))
    labels = np.asarray(inputs["labels"]).astype(np.int64)
    centers = np.ascontiguousarray(np.asarray(inputs["centers"], dtype=np.float32))
    assert x.shape == (B, D) and labels.shape == (B,) and centers.shape == (C, D)

    in_maps, counts, cr, mt = _shard(x, labels, centers)
    nc = _get_nc(cr, mt)
    res = run_bass_kernel_spmd(nc, in_maps, core_ids=list(range(N_CORES)))
    return _combine([r["partial"] for r in res.results], counts, mt)
